# revision 4
# baseline (speedup 1.0000x reference)
"""Causal single-head attention on 8 Trainium2 NeuronCores.

Problem: x[4, 2048, 1024], Wq/Wk/Wv[1024, 1024] (torch Linear layout).
  q = x @ Wq.T ; k = x @ Wk.T ; v = x @ Wv.T
  out = softmax(mask(q @ k.T) / 32) @ v

Sharding: 8 cores = (batch b = core // 2) x (query-parity h = core % 2).
Core (b, h) owns q-tiles t = 2j + h (j = 0..7) of batch b.  Parity
interleaving makes the per-slot causal span identical across cores
(slot j spans 256*(j+1) keys; the h-dependent diagonal is handled by
per-core mask data), so a single SPMD program serves all 8 cores.

Folded algebra (removes the K/V projections entirely, so no work is
duplicated across the two cores of a batch and no collectives needed):
  scores = Q K^T = Q Wk x^T      with Q' = (x_q Wq^T) Wk   [per-core]
  out    = A x Wv^T              with U  = A x             [per-slot]
Per-core tensor work: Qproj 65k + Q' 65k + scores 74k + transposes 17k
+ A@X 74k + U@Wv 66k ~= 361k PE cycles (~150us at 2.4 GHz), fully
balanced across cores.  All matmuls run in bf16 (1 cycle/row, f32
accumulate; ~3e-3 rel err).  Softmax skips the max-subtract (logits
are O(1) after the 1/32 scale); 1/denominator is folded into the
PSUM->SBUF eviction of U.
"""

import numpy as np

import concourse.mybir as mybir
import concourse.tile as tile
from concourse import bacc
from concourse.bass_utils import run_bass_kernel_spmd

P = 128
B = 4
S = 2048
D = 1024
ND = D // P          # d-tiles
NE = D // P          # e-tiles
NQ = 8               # query slots per core (128 rows each)
KC = 256             # score chunk width (keys)
NT = S // P          # key tiles
F32 = mybir.dt.float32
BF16 = mybir.dt.bfloat16

MASK_VAL = -1.0e5    # additive pre-scale mask; exp((s+MASK_VAL)/32) == 0.0

_CACHE: dict = {}
USE_CC = False


def build_program(reps: int = 1):
    nc = bacc.Bacc(None)

    xqT = nc.dram_tensor("xqT", [D, NQ * P], BF16, kind="ExternalInput")
    wq = nc.dram_tensor("wq", [D, D], BF16, kind="ExternalInput")    # Wq^T [d, e]
    wk = nc.dram_tensor("wk", [D, D], BF16, kind="ExternalInput")    # Wk   [e', d]
    wvT = nc.dram_tensor("wvT", [D, D], BF16, kind="ExternalInput")  # Wv^T [d, e]
    xT = nc.dram_tensor("xT", [D, S], BF16, kind="ExternalInput")    # x^T  [d, s]
    xr = nc.dram_tensor("xr", [S, D], BF16, kind="ExternalInput")    # x    [s, d]
    mask = nc.dram_tensor("mask", [NQ, P, KC], BF16, kind="ExternalInput")
    ident = nc.dram_tensor("ident", [P, P], BF16, kind="ExternalInput")
    out = nc.dram_tensor("out", [NQ * P, D], F32, kind="ExternalOutput")

    xqT_r = xqT[:].rearrange("(i p) q -> p i q", p=P)
    wq_r = wq[:].rearrange("(i p) e -> p i e", p=P)
    wk_r = wk[:].rearrange("(i p) d -> p i d", p=P)
    wvT_r = wvT[:].rearrange("(i p) e -> p i e", p=P)
    xT_r = xT[:].rearrange("(i p) s -> p i s", p=P)
    xr_r = xr[:].rearrange("(t p) d -> p t d", p=P)

    with tile.TileContext(nc) as tc:
      for _rep in range(reps):
        with tc.tile_pool(name="big", bufs=1) as bigp:
            # persistent attention-phase tensors
            QpT = bigp.tile([P, ND, NQ * P], BF16, tag="QpT")   # Q'^T [d, q]
            xT_s = bigp.tile([P, ND, S], BF16, tag="xT_s")      # x^T  [d, s]
            xr_s = bigp.tile([P, NT, D], BF16, tag="xr_s")      # x    [k, d]
            wvT_s = bigp.tile([P, ND, D], BF16, tag="wvT_s")    # Wv^T [d, e]
            mask_s = bigp.tile([P, NQ, KC], BF16, tag="mask")
            ident_s = bigp.tile([P, P], BF16, tag="ident")

            # ---- phase 1+2: Q^T = Wq x_q^T ; Q'^T = Wk^T Q^T ----
            with (
                tc.tile_pool(name="proj", bufs=1) as projp,
                tc.tile_pool(name="ps_p", bufs=2, space="PSUM") as pspp,
            ):
                wq_s = projp.tile([P, ND, D], BF16, tag="wq")
                xq_s = projp.tile([P, ND, NQ * P], BF16, tag="xq")
                wk_s = projp.tile([P, ND, D], BF16, tag="wk")
                Qt = projp.tile([P, NE, NQ * P], BF16, tag="Qt")

                # DMA queue order == need order
                for i in range(ND):
                    nc.sync.dma_start(wq_s[:, i : i + 1, :], wq_r[:, i : i + 1, :])
                for i in range(ND):
                    nc.sync.dma_start(xq_s[:, i : i + 1, :], xqT_r[:, i : i + 1, :])
                for i in range(ND):
                    nc.sync.dma_start(wk_s[:, i : i + 1, :], wk_r[:, i : i + 1, :])
                nc.sync.dma_start(ident_s[:], ident[:])
                nc.sync.dma_start(mask_s[:], mask[:].rearrange("j p k -> p j k"))
                # first half of keys, then wvT, then second half
                for i in range(ND):
                    nc.sync.dma_start(
                        xT_s[:, i : i + 1, 0 : S // 2], xT_r[:, i : i + 1, 0 : S // 2]
                    )
                for t in range(NT // 2):
                    nc.sync.dma_start(xr_s[:, t : t + 1, :], xr_r[:, t : t + 1, :])
                for i in range(ND):
                    nc.sync.dma_start(wvT_s[:, i : i + 1, :], wvT_r[:, i : i + 1, :])
                for i in range(ND):
                    nc.sync.dma_start(
                        xT_s[:, i : i + 1, S // 2 : S], xT_r[:, i : i + 1, S // 2 : S]
                    )
                for t in range(NT // 2, NT):
                    nc.sync.dma_start(xr_s[:, t : t + 1, :], xr_r[:, t : t + 1, :])

                # Q^T[e, q] = sum_d Wq^T[d, e] x_q^T[d, q]
                for qc in range(2):
                    for e in range(NE):
                        pq = pspp.tile([P, 512], F32, tag="pq")
                        for d in range(ND):
                            nc.tensor.matmul(
                                pq[:],
                                wq_s[:, d, e * P : (e + 1) * P],
                                xq_s[:, d, qc * 512 : (qc + 1) * 512],
                                start=(d == 0),
                                stop=(d == ND - 1),
                            )
                        nc.scalar.copy(Qt[:, e, qc * 512 : (qc + 1) * 512], pq[:])
                # Q'^T[d, q] = sum_e' Wk[e', d] Q^T[e', q]
                for qc in range(2):
                    for dt_ in range(ND):
                        pp = pspp.tile([P, 512], F32, tag="pp")
                        for e in range(NE):
                            nc.tensor.matmul(
                                pp[:],
                                wk_s[:, e, dt_ * P : (dt_ + 1) * P],
                                Qt[:, e, qc * 512 : (qc + 1) * 512],
                                start=(e == 0),
                                stop=(e == NE - 1),
                            )
                        nc.scalar.copy(QpT[:, dt_, qc * 512 : (qc + 1) * 512], pp[:])

            # ---- phase 3: attention ----
            with (
                tc.tile_pool(name="erow", bufs=2) as erowp,
                tc.tile_pool(name="et", bufs=6) as etp,
                tc.tile_pool(name="ut", bufs=3) as utp,
                tc.tile_pool(name="stat", bufs=2) as statp,
                tc.tile_pool(name="us", bufs=2) as usp,
                tc.tile_pool(name="orow", bufs=2) as orowp,
                tc.tile_pool(name="ps_s", bufs=2, space="PSUM") as pssp,
                tc.tile_pool(name="ps_t", bufs=2, space="PSUM") as pstp,
                tc.tile_pool(name="ps_u", bufs=2, space="PSUM") as psup,
                tc.tile_pool(name="ps_o", bufs=2, space="PSUM") as psop,
            ):
                pend = {}   # slot -> Us tile

                def transpose_groups(src, n_tiles, pool, tag):
                    """Transpose n_tiles 128-col tiles of src in groups of 4
                    sharing one PSUM bank; returns [(sbuf_tile, col_off)]."""
                    outs = []
                    for g0 in range(0, n_tiles, 4):
                        gsz = min(4, n_tiles - g0)
                        pt = pstp.tile([P, 512], BF16, tag="pt")
                        for i in range(gsz):
                            nc.tensor.transpose(
                                pt[:, i * P : (i + 1) * P],
                                src[:, (g0 + i) * P : (g0 + i + 1) * P],
                                ident_s[:],
                            )
                        tg = pool.tile([P, 512], BF16, tag=tag)
                        nc.vector.tensor_copy(
                            tg[:, 0 : gsz * P], pt[:, 0 : gsz * P]
                        )
                        outs += [(tg, i * P) for i in range(gsz)]
                    return outs

                def emit_finish(j):
                    """U^T transposes + U@Wv^T projection + store for slot j."""
                    Us = pend.pop(j)
                    uts = transpose_groups(Us, ND, utp, "ut")
                    pos = [
                        psop.tile([P, 512], F32, tag="po", name=f"po{j}_{eh}")
                        for eh in range(2)
                    ]
                    for dt_ in range(ND):
                        ut, off = uts[dt_]
                        for eh in range(2):
                            nc.tensor.matmul(
                                pos[eh][:],
                                ut[:, off : off + P],
                                wvT_s[:, dt_, eh * 512 : (eh + 1) * 512],
                                start=(dt_ == 0),
                                stop=(dt_ == ND - 1),
                            )
                    orow = orowp.tile([P, D], F32, tag="orow")
                    for eh in range(2):
                        nc.scalar.copy(orow[:, eh * 512 : (eh + 1) * 512], pos[eh][:])
                    nc.sync.dma_start(out[j * P : (j + 1) * P, :], orow[:])

                for j in range(NQ):
                    nk = j + 1          # 256-wide score chunks
                    nt = 2 * (j + 1)    # 128-wide key tiles
                    erow = erowp.tile([P, S], BF16, tag="erow")
                    partials = statp.tile([P, NQ], F32, tag="partials")
                    den = statp.tile([P, 1], F32, tag="den")
                    rcp = statp.tile([P, 1], F32, tag="rcp")

                    # scores + exp, chunk by chunk
                    for kc in range(nk):
                        ps = pssp.tile([P, KC], F32, tag="ps")
                        for d in range(ND):
                            nc.tensor.matmul(
                                ps[:],
                                QpT[:, d, j * P : (j + 1) * P],
                                xT_s[:, d, kc * KC : (kc + 1) * KC],
                                start=(d == 0),
                                stop=(d == ND - 1),
                            )
                        if kc == nk - 1:
                            nc.vector.tensor_add(ps[:], ps[:], mask_s[:, j, :])
                        nc.scalar.activation(
                            erow[:, kc * KC : (kc + 1) * KC],
                            ps[:],
                            mybir.ActivationFunctionType.Exp,
                            scale=float(1.0 / np.sqrt(D)),
                            accum_out=partials[:, kc : kc + 1],
                        )

                    # finish previous slot here: its PE work (U^T + U@Wv)
                    # hides this slot's exp tail and the DVE scale of U
                    if j > 0:
                        emit_finish(j - 1)

                    nc.vector.reduce_sum(
                        den[:], partials[:, :nk], axis=mybir.AxisListType.X
                    )
                    nc.vector.reciprocal(rcp[:], den[:])

                    # transpose exp-scores to [k, q]
                    ets = transpose_groups(erow, nt, etp, "et")

                    # U[q, d] = sum_k A^T[k, q].T x[k, d]
                    pus = [
                        psup.tile([P, 512], F32, tag="pu", name=f"pu{j}_{eh}")
                        for eh in range(2)
                    ]
                    for kt in range(nt):
                        et, off = ets[kt]
                        for eh in range(2):
                            nc.tensor.matmul(
                                pus[eh][:],
                                et[:, off : off + P],
                                xr_s[:, kt, eh * 512 : (eh + 1) * 512],
                                start=(kt == 0),
                                stop=(kt == nt - 1),
                            )
                    Us = usp.tile([P, D], BF16, tag="Us")
                    for eh in range(2):
                        nc.vector.tensor_scalar_mul(
                            Us[:, eh * 512 : (eh + 1) * 512], pus[eh][:], rcp[:]
                        )
                    pend[j] = Us
                emit_finish(NQ - 1)

    nc.finalize()
    return nc


def make_mask(h: int) -> np.ndarray:
    """Additive mask for the last 256 columns of each slot's span."""
    import ml_dtypes

    m = np.zeros((NQ, P, KC), dtype=ml_dtypes.bfloat16)
    rows = np.arange(P)[:, None]
    cols = np.arange(P)[None, :]
    tri = np.where(cols <= rows, 0.0, MASK_VAL).astype(ml_dtypes.bfloat16)
    for j in range(NQ):
        if h == 1:
            # q-tile 2j+1: first 128 cols fully valid, diagonal in last 128
            m[j, :, P:] = tri
        else:
            # q-tile 2j: diagonal in first 128 cols, last 128 fully padded
            m[j, :, :P] = tri
            m[j, :, P:] = MASK_VAL
    return m


def make_in_maps(x, Wq, Wk, Wv, cc=False):
    import ml_dtypes

    bf = ml_dtypes.bfloat16
    x = np.asarray(x, dtype=np.float32)
    wq_h = np.ascontiguousarray(np.asarray(Wq, dtype=np.float32).T.astype(bf))
    wk_h = np.ascontiguousarray(np.asarray(Wk, dtype=np.float32).astype(bf))
    wvT_h = np.ascontiguousarray(np.asarray(Wv, dtype=np.float32).T.astype(bf))
    ident = np.eye(P, dtype=bf)
    masks = [make_mask(0), make_mask(1)]
    in_maps = []
    for c in range(8):
        b, h = c // 2, c % 2
        xb = x[b]
        xT_h = np.ascontiguousarray(xb.T.astype(bf))               # [D, S]
        xq_h = np.ascontiguousarray(
            xT_h.reshape(D, S // P, P)[:, [2 * j + h for j in range(NQ)], :].reshape(
                D, NQ * P
            )
        )
        xr_h = np.ascontiguousarray(xb.astype(bf))                 # [S, D]
        in_maps.append(
            {
                "xqT": xq_h,
                "wq": wq_h,
                "wk": wk_h,
                "wvT": wvT_h,
                "xT": xT_h,
                "xr": xr_h,
                "mask": masks[h],
                "ident": ident,
            }
        )
    return in_maps


def gather_output(results) -> np.ndarray:
    out = np.empty((B, S, D), dtype=np.float32)
    for c in range(8):
        b, h = c // 2, c % 2
        oc = results[c]["out"]
        for j in range(NQ):
            t = 2 * j + h
            out[b, t * P : (t + 1) * P, :] = oc[j * P : (j + 1) * P, :]
    return out


def kernel(x, Wq, Wk, Wv):
    if "nc" not in _CACHE:
        _CACHE["nc"] = build_program()
    nc = _CACHE["nc"]
    in_maps = make_in_maps(x, Wq, Wk, Wv)
    res = run_bass_kernel_spmd(nc, in_maps, core_ids=list(range(8)))
    return gather_output(res.results)
